# revision 3
# baseline (speedup 1.0000x reference)
"""TransformerConv GNN (3 layers) on 8 Trainium2 NeuronCores.

Sharding: dst-node partition across 8 cores (6250 nodes each). Per core,
nodes are bin-packed into 50 blocks of <=128 nodes s.t. each block has
<=17*128 incoming edges. Edge phase per block: per-edge k||v rows are
fetched with indirect DMA from an AllGather'ed bf16 kv table; q is expanded
per-edge with a one-hot matmul; softmax runs without max-subtraction
(logits bounded); alpha-weighted aggregation and the softmax denominators
are computed in one accumulating one-hot matmul into PSUM.
"""
import numpy as np
import ml_dtypes

import concourse.bass as bass
import concourse.bacc as bacc
import concourse.tile as tile
from concourse import mybir
from concourse import bass_utils
from concourse.masks import make_identity

N, E, DIN, DH, H = 50000, 800000, 128, 32, 4
DQKV = H * DH                    # 128
NCORES = 8
NPC = N // NCORES                # 6250
NBLK = 50
NS = 128
SPC = NBLK * NS                  # 6400 slots per core
TPB = 17                         # edge tiles per block
CAP = TPB * 128                  # 2176 edge slots per block
NT = NBLK * TPB                  # 850 tiles per core

f32 = mybir.dt.float32
bf16 = mybir.dt.bfloat16
i32 = mybir.dt.int32
bfnp = ml_dtypes.bfloat16

_cache = {}


def preprocess(edge_index):
    src = np.asarray(edge_index[0]).astype(np.int64)
    dst = np.asarray(edge_index[1]).astype(np.int64)
    dst_core = dst // NPC
    slot_of_node = np.full(N, -1, np.int64)
    per_core = []
    for c in range(NCORES):
        m = dst_core == c
        es, ed = src[m], dst[m]
        ln = ed - c * NPC
        deg = np.bincount(ln, minlength=NPC)
        order = np.argsort(-deg, kind="stable")
        bload = np.zeros(NBLK, np.int64)
        bcnt = np.zeros(NBLK, np.int64)
        blk_of = np.full(NPC, -1, np.int64)
        slot_in = np.full(NPC, -1, np.int64)
        for nidx in order:
            feas = (bload + deg[nidx] <= CAP) & (bcnt < NS)
            assert feas.any(), f"bin packing failed on core {c}"
            cand = np.where(feas)[0]
            b = cand[np.argmin(bload[cand])]
            blk_of[nidx] = b
            slot_in[nidx] = bcnt[b]
            bload[b] += deg[nidx]
            bcnt[b] += 1
        nodes = np.arange(c * NPC, (c + 1) * NPC)
        slot_of_node[nodes] = blk_of * NS + slot_in
        per_core.append((es, ed, blk_of, slot_in))
    cores = []
    for c in range(NCORES):
        es, ed, blk_of, slot_in = per_core[c]
        ln = ed - c * NPC
        eb = blk_of[ln]
        eslot = slot_in[ln]
        gsid = (es // NPC) * SPC + slot_of_node[es]
        idx = np.zeros((128, NT), np.int32)
        oh = np.zeros((NBLK, 128, TPB, NS), bfnp)
        ohT = np.zeros((NBLK, NS, TPB, 128), bfnp)
        for b in range(NBLK):
            m = eb == b
            g = gsid[m]
            ds = eslot[m]
            n = len(g)
            pos = np.arange(n)
            t, p = pos // 128, pos % 128
            idx[p, b * TPB + t] = g.astype(np.int32)
            oh[b, p, t, ds] = 1.0
            ohT[b, ds, t, p] = 1.0
        cores.append(dict(idx=idx, oh=oh.reshape(NBLK, 128, CAP),
                          ohT=ohT.reshape(NBLK, NS, CAP)))
    return cores, slot_of_node


def build_nc():
    nc = bacc.Bacc("TRN2", target_bir_lowering=False, debug=False,
                   num_devices=NCORES)
    xT = nc.dram_tensor("xT", [128, SPC], f32, kind="ExternalInput")
    wcols = [512, 512, 416]
    w_in = [nc.dram_tensor(f"w{l}", [128, wcols[l]], f32, kind="ExternalInput")
            for l in range(3)]
    bqkv_in = [nc.dram_tensor(f"bqkv{l}", [128, 384], f32, kind="ExternalInput")
               for l in range(3)]
    sdims = [128, 128, 32]
    bs_in = [nc.dram_tensor(f"bs{l}", [128, sdims[l]], f32, kind="ExternalInput")
             for l in range(3)]
    oh_in = nc.dram_tensor("oh", [NBLK, 128, CAP], bf16, kind="ExternalInput")
    ohT_in = nc.dram_tensor("ohT", [NBLK, NS, CAP], bf16, kind="ExternalInput")
    idx_in = nc.dram_tensor("idx", [128, NT], i32, kind="ExternalInput")
    y = nc.dram_tensor("y", [SPC, DH], f32, kind="ExternalOutput")

    AX = mybir.AxisListType.X
    OP = mybir.AluOpType
    AF = mybir.ActivationFunctionType

    with tile.TileContext(nc) as tc:
        with (
            tc.tile_pool(name="const", bufs=1) as constp,
            tc.tile_pool(name="node", bufs=3) as nodep,
            tc.tile_pool(name="blk", bufs=2) as blkp,
            tc.tile_pool(name="kvt", bufs=24) as kvtp,
            tc.tile_pool(name="tmp", bufs=4) as tmpp,
            tc.tile_pool(name="psq", bufs=3, space="PSUM") as psq,
            tc.tile_pool(name="psagg", bufs=2, space="PSUM") as psagg,
            tc.tile_pool(name="psnode", bufs=2, space="PSUM") as psnode,
            tc.tile_pool(name="psT", bufs=1, space="PSUM") as psT,
            tc.tile_pool(name="dram", bufs=1, space="DRAM") as dram,
        ):
            ident = constp.tile([128, 128], f32)
            make_identity(nc, ident[:])
            idx_sb = constp.tile([128, NT], i32)
            nc.sync.dma_start(idx_sb[:], idx_in[:])
            w_sb, bqkv_sb, bs_sb = [], [], []
            for l in range(3):
                w = constp.tile([128, wcols[l]], f32, tag=f"w{l}")
                nc.sync.dma_start(w[:], w_in[l][:])
                w_sb.append(w)
                bq = constp.tile([128, 384], f32, tag=f"bq{l}")
                nc.sync.dma_start(bq[:], bqkv_in[l][:])
                bqkv_sb.append(bq)
                bs = constp.tile([128, sdims[l]], f32, tag=f"bs{l}")
                nc.sync.dma_start(bs[:], bs_in[l][:])
                bs_sb.append(bs)

            q_tab = dram.tile([SPC, DQKV], bf16)
            kv_loc = dram.tile([SPC, 2 * DQKV], bf16)
            kv_full = dram.tile([NCORES * SPC, 2 * DQKV], bf16)
            s_tab = dram.tile([SPC, 128], f32)
            hT1 = dram.tile([128, SPC], f32)
            hT2 = dram.tile([128, SPC], f32)

            hsrc = [xT, hT1, hT2]
            for l in range(3):
                ds = sdims[l]
                wc = wcols[l]
                # ---- node phase ----
                for b in range(NBLK):
                    cs = slice(b * NS, (b + 1) * NS)
                    hb = nodep.tile([128, 128], f32, tag="hb")
                    nc.sync.dma_start(hb[:], hsrc[l][:, cs])
                    ps = psnode.tile([128, wc], f32, tag="psn")
                    nc.tensor.matmul(ps[:], lhsT=hb[:], rhs=w_sb[l][:],
                                     start=True, stop=True)
                    qkv = nodep.tile([128, 384], bf16, tag="qkv")
                    nc.vector.tensor_tensor(qkv[:], ps[:, 0:384],
                                            bqkv_sb[l][:], op=OP.add)
                    ssb = nodep.tile([128, ds], f32, tag="ssb")
                    nc.vector.tensor_tensor(ssb[:], ps[:, 384:wc],
                                            bs_sb[l][:], op=OP.add)
                    nc.sync.dma_start(q_tab[cs, :], qkv[:, 0:128])
                    nc.sync.dma_start(kv_loc[cs, :], qkv[:, 128:384])
                    nc.sync.dma_start(s_tab[cs, 0:ds], ssb[:])
                nc.gpsimd.collective_compute(
                    "AllGather", OP.bypass,
                    replica_groups=[list(range(NCORES))],
                    ins=[kv_loc.opt()], outs=[kv_full.opt()],
                )
                # ---- edge phase ----
                for b in range(NBLK):
                    cs = slice(b * NS, (b + 1) * NS)
                    qb = blkp.tile([128, 128], bf16, tag="qb")
                    nc.sync.dma_start(qb[:], q_tab[cs, :])
                    ohb = blkp.tile([128, CAP], bf16, tag="ohb")
                    nc.sync.dma_start(ohb[:], oh_in[b])
                    ohTb = blkp.tile([128, CAP], bf16, tag="ohTb")
                    nc.sync.dma_start(ohTb[:], ohT_in[b])
                    st = blkp.tile([128, ds], f32, tag="st")
                    nc.sync.dma_start(st[:], s_tab[cs, 0:ds])
                    logits = blkp.tile([128, TPB * 4], f32, tag="logits")
                    msgb = blkp.tile([128, TPB * 132], bf16, tag="msgb")
                    oh3 = ohb[:].rearrange("p (t n) -> p t n", n=128)
                    ohT3 = ohTb[:].rearrange("p (t n) -> p t n", n=128)
                    msg3 = msgb[:].rearrange("p (t c) -> p t c", c=132)
                    kvts = []
                    for t in range(TPB):
                        g = b * TPB + t
                        kvt = kvtp.tile([128, 256], bf16, tag="kvt")
                        nc.gpsimd.indirect_dma_start(
                            out=kvt[:], out_offset=None,
                            in_=kv_full[:],
                            in_offset=bass.IndirectOffsetOnAxis(
                                ap=idx_sb[:, g:g + 1], axis=0),
                        )
                        kvts.append(kvt)
                        qe = psq.tile([128, 128], f32, tag="qe")
                        nc.tensor.matmul(qe[:], lhsT=ohT3[:, t, :], rhs=qb[:],
                                         start=True, stop=True)
                        tmp = tmpp.tile([128, 128], f32, tag="tmp")
                        nc.vector.tensor_tensor(tmp[:], qe[:], kvt[:, 0:128],
                                                op=OP.mult)
                        nc.vector.tensor_reduce(
                            logits[:, 4 * t:4 * t + 4],
                            tmp[:].rearrange("p (h d) -> p h d", d=DH),
                            axis=AX, op=OP.add)
                    nc.scalar.activation(
                        msg3[:, :, 128:132],
                        logits[:].rearrange("p (t h) -> p t h", h=4),
                        AF.Exp)
                    for t in range(TPB):
                        a_bc = (msg3[:, t, 128:132]
                                .rearrange("p (h o) -> p h o", o=1)
                                .to_broadcast([128, 4, DH]))
                        nc.vector.tensor_tensor(
                            msg3[:, t, 0:128].rearrange("p (h d) -> p h d", d=DH),
                            kvts[t][:, 128:256].rearrange("p (h d) -> p h d", d=DH),
                            a_bc, op=OP.mult)
                    pa = psagg.tile([128, 132], f32, tag="pa")
                    for t in range(TPB):
                        nc.tensor.matmul(pa[:], lhsT=oh3[:, t, :],
                                         rhs=msg3[:, t, :],
                                         start=(t == 0), stop=(t == TPB - 1))
                    rec = tmpp.tile([128, 4], f32, tag="rec")
                    nc.vector.tensor_scalar_add(rec[:], pa[:, 128:132], 1e-30)
                    nc.vector.reciprocal(rec[:], rec[:])
                    if l == 2:
                        nc.vector.tensor_scalar_mul(rec[:], rec[:], 1.0 / H)
                    outsb = tmpp.tile([128, 128], f32, tag="outsb")
                    rec_bc = (rec[:].rearrange("p (h o) -> p h o", o=1)
                              .to_broadcast([128, 4, DH]))
                    nc.vector.tensor_tensor(
                        outsb[:].rearrange("p (h d) -> p h d", d=DH),
                        pa[:, 0:128].rearrange("p (h d) -> p h d", d=DH),
                        rec_bc, op=OP.mult)
                    if l < 2:
                        nc.vector.tensor_tensor(outsb[:], outsb[:], st[:],
                                                op=OP.add)
                        hrow = tmpp.tile([128, 128], f32, tag="hrow")
                        nc.scalar.activation(hrow[:], outsb[:], AF.Relu)
                        pt = psT.tile([128, 128], f32, tag="pt")
                        nc.tensor.transpose(pt[:], hrow[:], ident[:])
                        hTs = tmpp.tile([128, 128], f32, tag="hTs")
                        nc.vector.tensor_copy(hTs[:], pt[:])
                        nxt = hT1 if l == 0 else hT2
                        nc.sync.dma_start(nxt[:, cs], hTs[:])
                    else:
                        mean = tmpp.tile([128, DH], f32, tag="mean")
                        nc.vector.tensor_reduce(
                            mean[:],
                            outsb[:].rearrange("p (h d) -> p d h", d=DH),
                            axis=AX, op=OP.add)
                        fin = tmpp.tile([128, DH], f32, tag="fin")
                        nc.vector.tensor_tensor(fin[:], mean[:], st[:],
                                                op=OP.add)
                        nc.sync.dma_start(y[cs, :], fin[:])
    nc.compile()
    return nc


def kernel(_trace=False, **inputs):
    x = np.asarray(inputs["x"], np.float32)
    cores, slot_of_node = preprocess(inputs["edge_index"])

    in_maps = []
    scale = 1.0 / np.sqrt(DH)
    wmats, bqkvs, bss = [], [], []
    for l in range(3):
        Wq = np.asarray(inputs[f"Wq{l}"], np.float32) * scale
        bq = np.asarray(inputs[f"bq{l}"], np.float32) * scale
        Wk = np.asarray(inputs[f"Wk{l}"], np.float32)
        bk = np.asarray(inputs[f"bk{l}"], np.float32)
        Wv = np.asarray(inputs[f"Wv{l}"], np.float32)
        bv = np.asarray(inputs[f"bv{l}"], np.float32)
        Ws = np.asarray(inputs[f"Ws{l}"], np.float32)
        bs = np.asarray(inputs[f"bs{l}"], np.float32)
        wmats.append(np.concatenate([Wq, Wk, Wv, Ws], axis=1).copy())
        bqkvs.append(np.tile(np.concatenate([bq, bk, bv])[None, :], (128, 1)).copy())
        bss.append(np.tile(bs[None, :], (128, 1)).copy())
    for c in range(NCORES):
        xTc = np.zeros((SPC, DIN), np.float32)
        nodes = np.arange(c * NPC, (c + 1) * NPC)
        xTc[slot_of_node[nodes]] = x[nodes]
        m = {"xT": xTc.T.copy(),
             "oh": cores[c]["oh"].astype(bfnp),
             "ohT": cores[c]["ohT"].astype(bfnp),
             "idx": cores[c]["idx"]}
        for l in range(3):
            m[f"w{l}"] = wmats[l]
            m[f"bqkv{l}"] = bqkvs[l]
            m[f"bs{l}"] = bss[l]
        in_maps.append(m)

    if "nc" not in _cache:
        _cache["nc"] = build_nc()
    nc = _cache["nc"]
    res = bass_utils.run_bass_kernel_spmd(nc, in_maps,
                                          core_ids=list(range(NCORES)),
                                          trace=_trace)
    _cache["last_result"] = res
    out = np.zeros((N, DH), np.float32)
    for c in range(NCORES):
        nodes = np.arange(c * NPC, (c + 1) * NPC)
        out[nodes] = res.results[c]["y"][slot_of_node[nodes]]
    return out
